# revision 3
# baseline (speedup 1.0000x reference)
"""Tropical min-max matmul kernel for Trainium2.

out[b, o] = min_i max(x[b, i], weight[i, o])   with  x: [1024, 512], weight: [512, 512], fp32.

Strategy
--------
Data-parallel over the batch dim: 8 NeuronCores x 128 rows of x each; weight
replicated (no collectives). Per core, the weight is held transposed
(wT[o, i], o on partitions in 4 row-blocks) so the contraction axis i is the
DVE free axis, and batch rows are processed in groups of G=16.

All device compute runs in fp16: the DVE's 2x_1p perf mode (2 elem/cyc/lane)
applies to tensor_tensor with 16-bit dtypes and unit innermost stride, while
tensor_reduce is capped at 1x for every dtype. So the min-reduction over i is
done as a log2 tree of tensor_tensor(min) ops (each at 2x) down to 8 elements,
with one final strided tensor_reduce(min). Inputs are rounded to fp16 on the
host (rel err <= ~5e-4, far inside the 2e-2 gate); min/max select values so
no further error accumulates.

Per group: one DMA partition-broadcast of 2 groups of x rows (double
buffered), one wide TT max over [128, 16*4*512], 6 tree TT mins, one reduce.
The per-core result lands as ot[128, 4*128] = [o-within-block, block*128+b];
the host reassembles it into out[b, o].
"""

import os
import sys

for _p in ("/opt/trn_rl_repo", "/root/.axon_site/_ro/trn_rl_repo"):
    if os.path.isdir(_p) and _p not in sys.path:
        sys.path.insert(0, _p)

import numpy as np

import concourse.bass as bass
import concourse.mybir as mybir
from concourse.bass_utils import run_bass_kernel_spmd

B, I, O = 1024, 512, 512
NCORES = 8
BS = B // NCORES   # 128 batch rows per core
NCH = I // 128     # 4 i-chunks
OBLK = O // 128    # 4 output-feature blocks

TRACE = False
LAST_RESULTS = None
BENCH = 0
BENCH_TIMES = None

_F32 = mybir.dt.float32
_F16 = mybir.dt.float16

# "fp16" (tree-min, 2x DVE modes) or "fp32" (exact, slow) or
# "fp16_flat" (fp16 but tensor_reduce instead of tree, for comparison)
DTYPE_MODE = os.environ.get("MINMAX_DTYPE", "fp16")


def _build_nc_wide(dt, detect_races=True, repeat=1, group=16, reduce_mode="tree"):
    """fp16 wide-group variant: GROUP batch rows per step.

    Per group: DMA partition-broadcast of x rows (two groups per DMA,
    double-buffered), one wide 2x-mode tensor_tensor(max) over
    [128, G*OBLK*I], then a min-tree of tensor_tensor(min) halvings
    (512 -> 8, each at 2x) and one small strided tensor_reduce(min).
    """
    nc = bass.Bass(detect_race_conditions=detect_races)
    G = group
    NGRP = BS // G
    W = G * NCH * I  # wide op free size per group (elements per partition)
    GO = G * OBLK    # number of (row, o-block) segments per group

    xd = nc.declare_dram_parameter("x", [BS, I], dt, isOutput=False)
    wt_d = nc.declare_dram_parameter("wT", [O, I], dt, isOutput=False)
    out_d = nc.declare_dram_parameter("ot", [128, OBLK * BS], dt, isOutput=True)

    x_rows = xd.rearrange("(g r) i -> g (r i)", r=G)  # [NGRP, G*I]

    with (
        nc.sbuf_tensor([128, OBLK * I], dt) as wt_sb,
        nc.sbuf_tensor([128, 2 * G * I], dt) as bc_sb,   # double-buffered bcast
        nc.sbuf_tensor([128, W], dt) as scr_sb,          # max results
        nc.sbuf_tensor([128, W // 2], dt) as scr2_sb,    # tree ping-pong
        nc.sbuf_tensor([128, OBLK * BS], dt) as ot_sb,
        nc.semaphore("dma_sem") as dma_sem,
        nc.semaphore("v_sem") as v_sem,
        nc.Block() as block,
    ):
        NB = repeat * NGRP

        def bc_tile(g):
            j = g % 2
            return bc_sb[:, j * G * I:(j + 1) * G * I]

        @block.sync
        def _(sync):
            sync.dma_start(
                out=wt_sb[:, :].rearrange("p (t i) -> p t i", t=OBLK),
                in_=wt_d.rearrange("(t p) i -> p t i", p=128),
            ).then_inc(dma_sem, 16)
            # broadcast x rows two groups at a time (one DMA fills both
            # halves of the double buffer)
            n_pair_dma = 0
            for gg in range(0, NB, 2):
                g = gg % NGRP
                if gg >= 2:
                    # both halves consumed by the TT-maxes of gg-2 and gg-1
                    sync.wait_ge(v_sem, 2 * gg - 1)
                src = x_rows[g:g + 2, :]
                src_b = bass.AP(
                    tensor=src.tensor,
                    offset=src.offset,
                    ap=[[0, 128], [G * I, 2], [1, G * I]],
                )
                sync.dma_start(out=bc_sb[:, :], in_=src_b).then_inc(dma_sem, 16)
                n_pair_dma += 1
            sync.wait_ge(v_sem, 2 * NB)
            sync.dma_start(out=out_d[:, :], in_=ot_sb[:, :]).then_inc(dma_sem, 16)
            sync.wait_ge(dma_sem, 16 * (n_pair_dma + 2))

        @block.vector
        def _(vector):
            wt_v = wt_sb[:, :]
            scr_v = scr_sb[:, :]
            scr2_v = scr2_sb[:, :]
            ps = scr_v.ap[0][0]
            p2 = scr2_v.ap[0][0]
            for gg in range(NB):
                g = gg % NGRP
                if gg % 2 == 0:
                    vector.wait_ge(dma_sem, 16 * (gg // 2 + 2))
                bc = bc_tile(gg)
                in0 = bass.AP(
                    tensor=wt_v.tensor, offset=wt_v.offset,
                    ap=[[wt_v.ap[0][0], 128], [0, G], [I, OBLK], [1, I]],
                )
                in1 = bass.AP(
                    tensor=bc.tensor, offset=bc.offset,
                    ap=[[bc.ap[0][0], 128], [I, G], [0, OBLK], [1, I]],
                )
                out = bass.AP(
                    tensor=scr_v.tensor, offset=scr_v.offset,
                    ap=[[ps, 128], [OBLK * I, G], [I, OBLK], [1, I]],
                )
                nc.vector.tensor_tensor(
                    out=out, in0=in0, in1=in1, op=mybir.AluOpType.max
                ).then_inc(v_sem, 1)

                ot_ap = ot_sb[:, :]
                red_out = bass.AP(
                    tensor=ot_ap.tensor,
                    offset=ot_ap.offset + g * G,
                    ap=[[ot_ap.ap[0][0], 128], [1, G], [BS, OBLK]],
                )

                if reduce_mode == "flat":
                    nc.vector.tensor_reduce(
                        out=red_out,
                        in_=out,
                        op=mybir.AluOpType.min,
                        axis=mybir.AxisListType.X,
                    ).then_inc(v_sem, 1)
                    continue

                # min tree: L 512 -> 8 via tensor_tensor(min) at 2x, ping-pong
                # between scr (compact [128, GO*L]) and scr2.
                L = I
                cur_t, cur_off, cur_p = scr_v.tensor, scr_v.offset, ps
                nxt_t, nxt_off, nxt_p = scr2_v.tensor, scr2_v.offset, p2
                while L > 8:
                    h = L // 2
                    i0 = bass.AP(
                        tensor=cur_t, offset=cur_off,
                        ap=[[cur_p, 128], [L, GO], [1, h]],
                    )
                    i1 = bass.AP(
                        tensor=cur_t, offset=cur_off + h,
                        ap=[[cur_p, 128], [L, GO], [1, h]],
                    )
                    o_ = bass.AP(
                        tensor=nxt_t, offset=nxt_off,
                        ap=[[nxt_p, 128], [h, GO], [1, h]],
                    )
                    nc.vector.tensor_tensor(
                        out=o_, in0=i0, in1=i1, op=mybir.AluOpType.min
                    )
                    cur_t, cur_off, cur_p, nxt_t, nxt_off, nxt_p = (
                        nxt_t, nxt_off, nxt_p, cur_t, cur_off, cur_p,
                    )
                    L = h
                # compact layout is [gr][oblk][L]; view as
                # [128, G, OBLK, L] with strides [OBLK*L, L, 1]
                red_in = bass.AP(
                    tensor=cur_t, offset=cur_off,
                    ap=[[cur_p, 128], [OBLK * L, G], [L, OBLK], [1, L]],
                )
                nc.vector.tensor_reduce(
                    out=red_out,
                    in_=red_in,
                    op=mybir.AluOpType.min,
                    axis=mybir.AxisListType.X,
                ).then_inc(v_sem, 1)

    return nc


_NC_CACHE = {}


def _get_nc(mode):
    if mode not in _NC_CACHE:
        if mode == "fp32":
            _NC_CACHE[mode] = _build_nc_wide(_F32, group=16, reduce_mode="flat")
        elif mode == "fp16_flat":
            _NC_CACHE[mode] = _build_nc_wide(_F16, group=16, reduce_mode="flat")
        else:
            _NC_CACHE[mode] = _build_nc_wide(_F16, group=16, reduce_mode="tree")
    return _NC_CACHE[mode]


def kernel(x, weight):
    global LAST_RESULTS
    x = np.asarray(x)
    weight = np.asarray(weight)
    in_dtype = x.dtype

    mode = DTYPE_MODE
    npdt = np.float32 if mode == "fp32" else np.float16
    nc = _get_nc(mode)

    wt_h = np.ascontiguousarray(weight.T.astype(npdt))  # [O, I]
    xh = x.astype(npdt)
    in_maps = [
        {
            "x": np.ascontiguousarray(xh[c * BS:(c + 1) * BS]),
            "wT": wt_h,
        }
        for c in range(NCORES)
    ]

    res = run_bass_kernel_spmd(nc, in_maps, list(range(NCORES)), trace=TRACE)
    LAST_RESULTS = res

    if BENCH > 0:
        import time as _time

        global BENCH_TIMES
        BENCH_TIMES = []
        for _ in range(BENCH):
            t0 = _time.perf_counter()
            run_bass_kernel_spmd(nc, in_maps, list(range(NCORES)), trace=False)
            BENCH_TIMES.append(_time.perf_counter() - t0)

    # ot[oo, t*BS + b] = out_core[b, t*128 + oo]
    parts = []
    for c in range(NCORES):
        ot = np.asarray(res.results[c]["ot"])          # [128, OBLK*BS]
        oc = ot.reshape(128, OBLK, BS).transpose(2, 1, 0).reshape(BS, O)
        parts.append(oc)
    out = np.concatenate(parts, axis=0)
    return out.astype(in_dtype)


# revision 5
# speedup vs baseline: 2.1385x; 2.1385x over previous
"""Tropical min-max matmul kernel for Trainium2.

out[b, o] = min_i max(x[b, i], weight[i, o])   with  x: [1024, 512], weight: [512, 512], fp32.

Strategy
--------
Data-parallel over the batch dim: 8 NeuronCores x 128 rows of x each; weight
replicated (no collectives). Per core, the weight is held transposed
(wT[o, i], o on partitions in 4 row-blocks) so the contraction axis i is the
DVE free axis. Batch rows are processed in uneven groups sized to the ISA's
65535 num_elem cap (31 rows -> FD = 31*4*512 = 63488), minimizing the DVE
instruction count: 5 groups -> 5 wide tensor_tensor(max) + 5 tensor_reduce(min)
instructions per core, plus broadcast DMAs (double buffered).

All device compute runs in fp16 (inputs rounded on the host; min/max select
values so rel err stays ~5e-4, far inside the 2e-2 gate).

The per-core result lands as ot[128, 4*128] = [o-within-block, block*128+b];
the host reassembles it into out[b, o].
"""

import os
import sys

for _p in ("/opt/trn_rl_repo", "/root/.axon_site/_ro/trn_rl_repo"):
    if os.path.isdir(_p) and _p not in sys.path:
        sys.path.insert(0, _p)

import numpy as np

import concourse.bass as bass
import concourse.mybir as mybir
from concourse.bass_utils import run_bass_kernel_spmd

B, I, O = 1024, 512, 512
NCORES = 8
BS = B // NCORES   # 128 batch rows per core
OBLK = O // 128    # 4 output-feature blocks

TRACE = False
LAST_RESULTS = None
BENCH = 0
BENCH_TIMES = None

_F32 = mybir.dt.float32
_F16 = mybir.dt.float16

# "fp16_g31" (default, 10 DVE insts) / "fp16_g16" (16 insts) / "fp32"
DTYPE_MODE = os.environ.get("MINMAX_DTYPE", "fp16_g31")

_GROUPS = {
    "fp16_g31": [31, 31, 31, 31, 4],
    "fp16_g16": [16] * 8,
    "fp32": [16] * 8,
}


def _build_nc(dt, groups, detect_races=True, repeat=1):
    nc = bass.Bass(detect_race_conditions=detect_races)
    NG = len(groups)
    gmax = max(groups)
    row0 = [sum(groups[:k]) for k in range(NG)]

    xd = nc.declare_dram_parameter("x", [BS, I], dt, isOutput=False)
    wt_d = nc.declare_dram_parameter("wT", [O, I], dt, isOutput=False)
    out_d = nc.declare_dram_parameter("ot", [128, OBLK * BS], dt, isOutput=True)

    with (
        nc.sbuf_tensor([128, OBLK * I], dt) as wt_sb,
        nc.sbuf_tensor([128, 2 * gmax * I], dt) as bc_sb,
        nc.sbuf_tensor([128, gmax * OBLK * I], dt) as scr_sb,
        nc.sbuf_tensor([128, OBLK * BS], dt) as ot_sb,
        nc.semaphore("dma_sem") as dma_sem,
        nc.semaphore("v_sem") as v_sem,
        nc.Block() as block,
    ):
        NB = repeat * NG

        def gidx(j):
            return j % NG

        @block.sync
        def _(sync):
            sync.dma_start(
                out=wt_sb[:, :].rearrange("p (t i) -> p t i", t=OBLK),
                in_=wt_d.rearrange("(t p) i -> p t i", p=128),
            ).then_inc(dma_sem, 16)
            for j in range(NB):
                k = gidx(j)
                Gg = groups[k]
                if j >= 2:
                    # buffer j%2 free once group j-2's TT max consumed it
                    sync.wait_ge(v_sem, 2 * (j - 2) + 1)
                xv = xd[:, :]
                src = bass.AP(
                    tensor=xv.tensor,
                    offset=xv.offset + row0[k] * I,
                    ap=[[0, 128], [1, Gg * I]],
                )
                buf = bc_sb[:, (j % 2) * gmax * I:(j % 2) * gmax * I + Gg * I]
                sync.dma_start(out=buf, in_=src).then_inc(dma_sem, 16)
            sync.wait_ge(v_sem, 2 * NB)
            sync.dma_start(out=out_d[:, :], in_=ot_sb[:, :]).then_inc(dma_sem, 16)
            sync.wait_ge(dma_sem, 16 * (NB + 2))

        @block.vector
        def _(vector):
            wt_v = wt_sb[:, :]
            scr_v = scr_sb[:, :]
            ps = scr_v.ap[0][0]
            for j in range(NB):
                k = gidx(j)
                Gg = groups[k]
                vector.wait_ge(dma_sem, 16 * (j + 2))
                bc = bc_sb[:, (j % 2) * gmax * I:(j % 2) * gmax * I + Gg * I]
                in0 = bass.AP(
                    tensor=wt_v.tensor, offset=wt_v.offset,
                    ap=[[wt_v.ap[0][0], 128], [0, Gg], [I, OBLK], [1, I]],
                )
                in1 = bass.AP(
                    tensor=bc.tensor, offset=bc.offset,
                    ap=[[bc.ap[0][0], 128], [I, Gg], [0, OBLK], [1, I]],
                )
                out = bass.AP(
                    tensor=scr_v.tensor, offset=scr_v.offset,
                    ap=[[ps, 128], [OBLK * I, Gg], [I, OBLK], [1, I]],
                )
                nc.vector.tensor_tensor(
                    out=out, in0=in0, in1=in1, op=mybir.AluOpType.max
                ).then_inc(v_sem, 1)

                ot_ap = ot_sb[:, :]
                red_out = bass.AP(
                    tensor=ot_ap.tensor,
                    offset=ot_ap.offset + row0[k],
                    ap=[[ot_ap.ap[0][0], 128], [1, Gg], [BS, OBLK]],
                )
                red_in = bass.AP(
                    tensor=scr_v.tensor, offset=scr_v.offset,
                    ap=[[ps, 128], [OBLK * I, Gg], [I, OBLK], [1, I]],
                )
                nc.vector.tensor_reduce(
                    out=red_out, in_=red_in,
                    op=mybir.AluOpType.min, axis=mybir.AxisListType.X,
                ).then_inc(v_sem, 1)

    return nc


def build_for_timing(repeat=1, mode=None):
    mode = mode or DTYPE_MODE
    dt = _F32 if mode == "fp32" else _F16
    return _build_nc(dt, _GROUPS[mode], repeat=repeat)


_NC_CACHE = {}


def _get_nc(mode):
    if mode not in _NC_CACHE:
        _NC_CACHE[mode] = build_for_timing(repeat=1, mode=mode)
    return _NC_CACHE[mode]


def kernel(x, weight):
    global LAST_RESULTS
    x = np.asarray(x)
    weight = np.asarray(weight)
    in_dtype = x.dtype

    mode = DTYPE_MODE
    npdt = np.float32 if mode == "fp32" else np.float16
    nc = _get_nc(mode)

    wt_h = np.ascontiguousarray(weight.T.astype(npdt))  # [O, I]
    xh = x.astype(npdt)
    in_maps = [
        {
            "x": np.ascontiguousarray(xh[c * BS:(c + 1) * BS]),
            "wT": wt_h,
        }
        for c in range(NCORES)
    ]

    res = run_bass_kernel_spmd(nc, in_maps, list(range(NCORES)), trace=TRACE)
    LAST_RESULTS = res

    if BENCH > 0:
        import time as _time

        global BENCH_TIMES
        BENCH_TIMES = []
        for _ in range(BENCH):
            t0 = _time.perf_counter()
            run_bass_kernel_spmd(nc, in_maps, list(range(NCORES)), trace=False)
            BENCH_TIMES.append(_time.perf_counter() - t0)

    # ot[oo, t*BS + b] = out_core[b, t*128 + oo]
    parts = []
    for c in range(NCORES):
        ot = np.asarray(res.results[c]["ot"])          # [128, OBLK*BS]
        oc = ot.reshape(128, OBLK, BS).transpose(2, 1, 0).reshape(BS, O)
        parts.append(oc)
    out = np.concatenate(parts, axis=0)
    return out.astype(in_dtype)


# revision 19
# speedup vs baseline: 2.4313x; 1.1369x over previous
"""Tropical min-max matmul kernel for Trainium2.

out[b, o] = min_i max(x[b, i], weight[i, o])   with  x: [1024, 512], weight: [512, 512], fp32.

Strategy
--------
Data-parallel over the batch dim: 8 NeuronCores x 128 rows of x each; weight
replicated (no collectives). Per core, the weight is held transposed
(wT[o, i], o on partitions in 4 row-blocks) so the contraction axis i is the
DVE free axis. Batch rows are processed in uneven groups sized to the ISA's
65535 num_elem cap (31 rows -> FD = 31*4*512 = 63488), minimizing the DVE
instruction count: 5 groups -> 5 wide tensor_tensor(max) + 5 tensor_reduce(min)
instructions per core, plus broadcast DMAs (double buffered).

All device compute runs in fp16 (inputs rounded on the host; min/max select
values so rel err stays ~5e-4, far inside the 2e-2 gate).

The per-core result lands as ot[128, 4*128] = [o-within-block, block*128+b];
the host reassembles it into out[b, o].
"""

import os
import sys

for _p in ("/opt/trn_rl_repo", "/root/.axon_site/_ro/trn_rl_repo"):
    if os.path.isdir(_p) and _p not in sys.path:
        sys.path.insert(0, _p)

import numpy as np

import concourse.bass as bass
import concourse.mybir as mybir
from concourse.bass_utils import run_bass_kernel_spmd

B, I, O = 1024, 512, 512
NCORES = 8
BS = B // NCORES   # 128 batch rows per core
OBLK = O // 128    # 4 output-feature blocks

TRACE = False
LAST_RESULTS = None
BENCH = 0
BENCH_TIMES = None

_F32 = mybir.dt.float32
_F16 = mybir.dt.float16

# "fp16_g31" (default) / "fp16_g28" / "fp16_g16" / "fp32"
DTYPE_MODE = os.environ.get("MINMAX_DTYPE", "fp16_g31")

_GROUPS = {
    # mode: (groups, nbuf).  Per-instruction total free size caps at 65535
    # (walrus num_elem); SBUF adds scr(4*gmax KB) + nbuf*gmax KB + 5KB
    # <= ~207KB.  So gmax <= 31 and 5 groups minimum.
    "fp16_g28": ([28, 28, 28, 28, 16], 3),
    "fp16_g31": ([31, 31, 31, 31, 4], 2),
    "fp16_g16": ([16] * 8, 4),
    "fp32": ([16] * 8, 2),
}


def _build_nc(dt, groups, detect_races=True, repeat=1, nbuf=4):
    nc = bass.Bass(detect_race_conditions=detect_races)
    NG = len(groups)
    gmax = max(groups)
    row0 = [sum(groups[:k]) for k in range(NG)]
    # vector waits on DMA completion cost one queue entry (~30us) each, so
    # issue one wait covering the next WSPAN groups instead of one per
    # group.  bc buffer j%nbuf is recycled once group j-nbuf is reduced;
    # WSPAN = nbuf-1 leaves one full group of slack before the furthest
    # covered DMA is needed (no just-in-time stall).
    WSPAN = max(1, nbuf - 1)

    xd = nc.declare_dram_parameter("x", [BS, I], dt, isOutput=False)
    wt_d = nc.declare_dram_parameter("wT", [O, I], dt, isOutput=False)
    out_d = nc.declare_dram_parameter("ot", [128, OBLK * BS], dt, isOutput=True)

    with (
        nc.sbuf_tensor([128, OBLK * I], dt) as wt_sb,
        nc.sbuf_tensor([128, nbuf * gmax * I], dt) as bc_sb,
        nc.sbuf_tensor([128, gmax * OBLK * I], dt) as scr_sb,
        nc.sbuf_tensor([128, OBLK * BS], dt) as ot_sb,
        nc.semaphore("dma_sem") as dma_sem,
        nc.semaphore("v_sem") as v_sem,
        nc.Block() as block,
    ):
        NB = repeat * NG

        def gidx(j):
            return j % NG

        @block.sync
        def _(sync):
            sync.dma_start(
                out=wt_sb[:, :].rearrange("p (t i) -> p t i", t=OBLK),
                in_=wt_d.rearrange("(t p) i -> p t i", p=128),
            ).then_inc(dma_sem, 16)
            for j in range(NB):
                k = gidx(j)
                Gg = groups[k]
                if j >= nbuf:
                    # buffer j%nbuf free once group j-nbuf is fully reduced
                    # (the reduce trails its TT in DVE program order)
                    sync.wait_ge(v_sem, j - nbuf + 1)
                xv = xd[:, :]
                src = bass.AP(
                    tensor=xv.tensor,
                    offset=xv.offset + row0[k] * I,
                    ap=[[0, 128], [1, Gg * I]],
                )
                s0 = (j % nbuf) * gmax * I
                buf = bc_sb[:, s0:s0 + Gg * I]
                sync.dma_start(out=buf, in_=src).then_inc(dma_sem, 16)
            sync.wait_ge(v_sem, NB)
            sync.dma_start(out=out_d[:, :], in_=ot_sb[:, :]).then_inc(dma_sem, 16)
            sync.wait_ge(dma_sem, 16 * (NB + 2))

        @block.vector
        def _(vector):
            wt_v = wt_sb[:, :]
            scr_v = scr_sb[:, :]
            ps = scr_v.ap[0][0]
            for j in range(NB):
                k = gidx(j)
                Gg = groups[k]
                if j % WSPAN == 0:
                    # one wait covers bc DMAs for groups j .. j+WSPAN-1
                    jc = min(j + WSPAN - 1, NB - 1)
                    vector.wait_ge(dma_sem, 16 * (jc + 2))
                s0 = (j % nbuf) * gmax * I
                bc = bc_sb[:, s0:s0 + Gg * I]
                in0 = bass.AP(
                    tensor=wt_v.tensor, offset=wt_v.offset,
                    ap=[[wt_v.ap[0][0], 128], [0, Gg], [I, OBLK], [1, I]],
                )
                in1 = bass.AP(
                    tensor=bc.tensor, offset=bc.offset,
                    ap=[[bc.ap[0][0], 128], [I, Gg], [0, OBLK], [1, I]],
                )
                out = bass.AP(
                    tensor=scr_v.tensor, offset=scr_v.offset,
                    ap=[[ps, 128], [OBLK * I, Gg], [I, OBLK], [1, I]],
                )
                nc.vector.tensor_tensor(
                    out=out, in0=in0, in1=in1, op=mybir.AluOpType.max
                )

                ot_ap = ot_sb[:, :]
                red_out = bass.AP(
                    tensor=ot_ap.tensor,
                    offset=ot_ap.offset + row0[k],
                    ap=[[ot_ap.ap[0][0], 128], [1, Gg], [BS, OBLK]],
                )
                red_in = bass.AP(
                    tensor=scr_v.tensor, offset=scr_v.offset,
                    ap=[[ps, 128], [OBLK * I, Gg], [I, OBLK], [1, I]],
                )
                nc.vector.tensor_reduce(
                    out=red_out, in_=red_in,
                    op=mybir.AluOpType.min, axis=mybir.AxisListType.X,
                ).then_inc(v_sem, 1)

    return nc


def build_for_timing(repeat=1, mode=None):
    mode = mode or DTYPE_MODE
    dt = _F32 if mode == "fp32" else _F16
    groups, nbuf = _GROUPS[mode]
    return _build_nc(dt, groups, repeat=repeat, nbuf=nbuf)


_NC_CACHE = {}


def _get_nc(mode):
    if mode not in _NC_CACHE:
        _NC_CACHE[mode] = build_for_timing(repeat=1, mode=mode)
    return _NC_CACHE[mode]


def kernel(x, weight):
    global LAST_RESULTS
    x = np.asarray(x)
    weight = np.asarray(weight)
    in_dtype = x.dtype

    mode = DTYPE_MODE
    npdt = np.float32 if mode == "fp32" else np.float16
    nc = _get_nc(mode)

    wt_h = np.ascontiguousarray(weight.T.astype(npdt))  # [O, I]
    xh = x.astype(npdt)
    in_maps = [
        {
            "x": np.ascontiguousarray(xh[c * BS:(c + 1) * BS]),
            "wT": wt_h,
        }
        for c in range(NCORES)
    ]

    res = run_bass_kernel_spmd(nc, in_maps, list(range(NCORES)), trace=TRACE)
    LAST_RESULTS = res

    if BENCH > 0:
        import time as _time

        global BENCH_TIMES
        BENCH_TIMES = []
        for _ in range(BENCH):
            t0 = _time.perf_counter()
            run_bass_kernel_spmd(nc, in_maps, list(range(NCORES)), trace=False)
            BENCH_TIMES.append(_time.perf_counter() - t0)

    # ot[oo, t*BS + b] = out_core[b, t*128 + oo]
    parts = []
    for c in range(NCORES):
        ot = np.asarray(res.results[c]["ot"])          # [128, OBLK*BS]
        oc = ot.reshape(128, OBLK, BS).transpose(2, 1, 0).reshape(BS, O)
        parts.append(oc)
    out = np.concatenate(parts, axis=0)
    return out.astype(in_dtype)


# revision 21
# speedup vs baseline: 3.1260x; 1.2857x over previous
"""Tropical min-max matmul kernel for Trainium2.

out[b, o] = min_i max(x[b, i], weight[i, o])   with  x: [1024, 512], weight: [512, 512], fp32.

Strategy
--------
Data-parallel over the batch dim: 8 NeuronCores x 128 rows of x each; weight
replicated (no collectives). Per core, the weight is held transposed
(wT[o, i], o on partitions in 4 row-blocks) so the contraction axis i is the
DVE free axis. Batch rows are processed in uneven groups sized to the ISA's
65535 num_elem cap (31 rows -> FD = 31*4*512 = 63488), minimizing the DVE
instruction count: 5 groups -> 5 wide tensor_tensor(max) + 5 tensor_reduce(min)
instructions per core, plus broadcast DMAs (double buffered).

All device compute runs in fp16 (inputs rounded on the host; min/max select
values so rel err stays ~5e-4, far inside the 2e-2 gate).

The per-core result lands as ot[128, 4*128] = [o-within-block, block*128+b];
the host reassembles it into out[b, o].
"""

import os
import sys

for _p in ("/opt/trn_rl_repo", "/root/.axon_site/_ro/trn_rl_repo"):
    if os.path.isdir(_p) and _p not in sys.path:
        sys.path.insert(0, _p)

import numpy as np

import concourse.bass as bass
import concourse.mybir as mybir
from concourse.bass_utils import run_bass_kernel_spmd

B, I, O = 1024, 512, 512
NCORES = 8
BS = B // NCORES   # 128 batch rows per core
OBLK = O // 128    # 4 output-feature blocks

TRACE = False
LAST_RESULTS = None
BENCH = 0
BENCH_TIMES = None

_F32 = mybir.dt.float32
_F16 = mybir.dt.float16

# "fp16_g31" (default) / "fp16_g28" / "fp16_g16" / "fp32"
DTYPE_MODE = os.environ.get("MINMAX_DTYPE", "fp16_g31")

_GROUPS = {
    # mode: (groups, nbuf).  Per-instruction total free size caps at 65535
    # (walrus num_elem); SBUF adds scr(4*gmax KB) + nbuf*gmax KB + 5KB
    # <= ~207KB.  So gmax <= 31 and 5 groups minimum.
    "fp16_g28": ([28, 28, 28, 28, 16], 3),
    "fp16_g31": ([31, 31, 31, 31, 4], 2),
    "fp16_g16": ([16] * 8, 4),
    "fp32": ([16] * 8, 2),
}


def _build_nc(dt, groups, detect_races=True, repeat=1, nbuf=4):
    nc = bass.Bass(detect_race_conditions=detect_races)
    NG = len(groups)
    gmax = max(groups)
    row0 = [sum(groups[:k]) for k in range(NG)]
    # vector waits on DMA completion cost one queue entry (~30us) each, so
    # issue one wait covering the next WSPAN groups instead of one per
    # group.  bc buffer j%nbuf is recycled once group j-nbuf is reduced;
    # WSPAN = nbuf-1 leaves one full group of slack before the furthest
    # covered DMA is needed (no just-in-time stall).
    WSPAN = max(1, nbuf - 1)

    xd = nc.declare_dram_parameter("x", [BS, I], dt, isOutput=False)
    wt_d = nc.declare_dram_parameter("wT", [O, I], dt, isOutput=False)
    out_d = nc.declare_dram_parameter("ot", [128, OBLK * BS], dt, isOutput=True)

    with (
        nc.sbuf_tensor([128, OBLK * I], dt) as wt_sb,
        nc.sbuf_tensor([128, nbuf * gmax * I], dt) as bc_sb,
        nc.sbuf_tensor([128, gmax * OBLK * I], dt) as scr_sb,
        nc.sbuf_tensor([128, OBLK * BS], dt) as ot_sb,
        nc.semaphore("dma_sem") as dma_sem,
        nc.semaphore("v_sem") as v_sem,
        nc.Block() as block,
    ):
        NB = repeat * NG

        def gidx(j):
            return j % NG

        @block.sync
        def _(sync):
            sync.dma_start(
                out=wt_sb[:, :].rearrange("p (t i) -> p t i", t=OBLK),
                in_=wt_d.rearrange("(t p) i -> p t i", p=128),
            ).then_inc(dma_sem, 16)
            for j in range(NB):
                k = gidx(j)
                Gg = groups[k]
                xv = xd[:, :]
                src = bass.AP(
                    tensor=xv.tensor,
                    offset=xv.offset + row0[k] * I,
                    ap=[[0, 128], [1, Gg * I]],
                )
                s0 = (j % nbuf) * gmax * I
                buf = bc_sb[:, s0:s0 + Gg * I]
                d = sync.dma_start(out=buf, in_=src)
                if j >= nbuf:
                    # buffer j%nbuf free once group j-nbuf is fully reduced
                    # (the reduce trails its TT in DVE program order); the
                    # wait rides the DMA instruction header (no extra queue
                    # entry)
                    d._wait_ge(v_sem, j - nbuf + 1)
                d.then_inc(dma_sem, 16)
            sync.dma_start(out=out_d[:, :], in_=ot_sb[:, :])._wait_ge(
                v_sem, NB).then_inc(dma_sem, 16)
            sync.wait_ge(dma_sem, 16 * (NB + 2))

        @block.vector
        def _(vector):
            wt_v = wt_sb[:, :]
            scr_v = scr_sb[:, :]
            ps = scr_v.ap[0][0]
            for j in range(NB):
                k = gidx(j)
                Gg = groups[k]
                s0 = (j % nbuf) * gmax * I
                bc = bc_sb[:, s0:s0 + Gg * I]
                in0 = bass.AP(
                    tensor=wt_v.tensor, offset=wt_v.offset,
                    ap=[[wt_v.ap[0][0], 128], [0, Gg], [I, OBLK], [1, I]],
                )
                in1 = bass.AP(
                    tensor=bc.tensor, offset=bc.offset,
                    ap=[[bc.ap[0][0], 128], [I, Gg], [0, OBLK], [1, I]],
                )
                out = bass.AP(
                    tensor=scr_v.tensor, offset=scr_v.offset,
                    ap=[[ps, 128], [OBLK * I, Gg], [I, OBLK], [1, I]],
                )
                # the wait for this group's broadcast DMA rides the TT's
                # instruction header (a standalone wait_ge costs a ~30us
                # queue entry in this environment)
                nc.vector.tensor_tensor(
                    out=out, in0=in0, in1=in1, op=mybir.AluOpType.max
                )._wait_ge(dma_sem, 16 * (j + 2))

                ot_ap = ot_sb[:, :]
                red_out = bass.AP(
                    tensor=ot_ap.tensor,
                    offset=ot_ap.offset + row0[k],
                    ap=[[ot_ap.ap[0][0], 128], [1, Gg], [BS, OBLK]],
                )
                red_in = bass.AP(
                    tensor=scr_v.tensor, offset=scr_v.offset,
                    ap=[[ps, 128], [OBLK * I, Gg], [I, OBLK], [1, I]],
                )
                nc.vector.tensor_reduce(
                    out=red_out, in_=red_in,
                    op=mybir.AluOpType.min, axis=mybir.AxisListType.X,
                ).then_inc(v_sem, 1)

    return nc


def build_for_timing(repeat=1, mode=None):
    mode = mode or DTYPE_MODE
    dt = _F32 if mode == "fp32" else _F16
    groups, nbuf = _GROUPS[mode]
    return _build_nc(dt, groups, repeat=repeat, nbuf=nbuf)


_NC_CACHE = {}


def _get_nc(mode):
    if mode not in _NC_CACHE:
        _NC_CACHE[mode] = build_for_timing(repeat=1, mode=mode)
    return _NC_CACHE[mode]


def kernel(x, weight):
    global LAST_RESULTS
    x = np.asarray(x)
    weight = np.asarray(weight)
    in_dtype = x.dtype

    mode = DTYPE_MODE
    npdt = np.float32 if mode == "fp32" else np.float16
    nc = _get_nc(mode)

    wt_h = np.ascontiguousarray(weight.T.astype(npdt))  # [O, I]
    xh = x.astype(npdt)
    in_maps = [
        {
            "x": np.ascontiguousarray(xh[c * BS:(c + 1) * BS]),
            "wT": wt_h,
        }
        for c in range(NCORES)
    ]

    res = run_bass_kernel_spmd(nc, in_maps, list(range(NCORES)), trace=TRACE)
    LAST_RESULTS = res

    if BENCH > 0:
        import time as _time

        global BENCH_TIMES
        BENCH_TIMES = []
        for _ in range(BENCH):
            t0 = _time.perf_counter()
            run_bass_kernel_spmd(nc, in_maps, list(range(NCORES)), trace=False)
            BENCH_TIMES.append(_time.perf_counter() - t0)

    # ot[oo, t*BS + b] = out_core[b, t*128 + oo]
    parts = []
    for c in range(NCORES):
        ot = np.asarray(res.results[c]["ot"])          # [128, OBLK*BS]
        oc = ot.reshape(128, OBLK, BS).transpose(2, 1, 0).reshape(BS, O)
        parts.append(oc)
    out = np.concatenate(parts, axis=0)
    return out.astype(in_dtype)
